# revision 12
# baseline (speedup 1.0000x reference)
"""Trainium2 Bass kernel for per-channel attention (nn_Attention_11690900979891).

Math (per batch b, channel d; H=256 positions, W=1):
    q,k,v = (qkv_w @ x_b + qkv_b) split              # each [512, 256]
    attn[h,g] = softmax_g(s*q[d,h]*k[d,g] + bias[h,g])
    attnout[d,h] = sum_g attn[h,g] * v[d,g]
    out_b = proj_w @ attnout + proj_b

Since |s*q*k| <= ~0.75, exp(s*q*k) is replaced by a degree-5 Chebyshev
polynomial; exp(z) ~= sum_m c_m z^m turns the softmax numerator/denominator
into dense GEMMs against EB = exp(bias):
    N[d,h] = sum_m c_m q[d,h]^m * (EB @ (v_d k_d^m))[h]
    D[d,h] = sum_m c_m q[d,h]^m * (EB @ (k_d^m))[h]
    attnout = N / D
so no transcendentals on the [256,256]-per-channel attention maps.

Sharding: core = (b, j); b = core//4, channels d in [128*j, 128*(j+1)).
Each core computes QKV + poly attention for its 128 channels, AllGathers
attnout within its 4-core batch group, then computes proj rows
[128*j : 128*(j+1)] of the output. Host only slices inputs / concatenates
outputs.
"""

import numpy as np

import concourse.bass as bass
import concourse.bacc as bacc
import concourse.mybir as mybir
from concourse import tile
from concourse.bass_utils import run_bass_kernel_spmd

F32 = mybir.dt.float32
F32R = mybir.dt.float32r
F16 = mybir.dt.float16

B, C, H = 2, 512, 256
NCORES = 8
GROUP = 4          # cores per batch
DLOC = C // GROUP  # 128 channels per core
SCALE = C ** -0.5
DEG = 5            # polynomial degree
POLY_A = 1.1       # fit domain [-A, A] for exp()

WS = 16
NTAB = (2 * WS - 1) ** 2


def _poly_coeffs():
    from numpy.polynomial import chebyshev as _ch
    c = _ch.Chebyshev.interpolate(np.exp, DEG, domain=[-POLY_A, POLY_A])
    return [float(v) for v in c.convert(kind=np.polynomial.Polynomial).coef]


COEF = _poly_coeffs()


def _rel_pos_index():
    coords = np.stack(
        np.meshgrid(np.arange(WS), np.arange(WS), indexing="ij"), 0
    ).reshape(2, -1)
    rel = coords[:, :, None] - coords[:, None, :]
    return np.mod(rel.transpose(1, 2, 0).sum(-1), NTAB).reshape(-1)


RPI = _rel_pos_index()


def build_nc(stage="full"):
    nc = bacc.Bacc(None, target_bir_lowering=False)

    xw = nc.declare_dram_parameter("xw", [C, 768], F32R, isOutput=False)
    biasT = nc.declare_dram_parameter("biasT", [H, H], F32, isOutput=False)
    bkv = nc.declare_dram_parameter("bkv", [128, 256], F32, isOutput=False)
    qpb = nc.declare_dram_parameter("qpb", [128, 2], F32, isOutput=False)
    out = nc.declare_dram_parameter("out", [DLOC, H], F32, isOutput=True)

    f32r = lambda ap: ap.bitcast(F32R)

    with tile.TileContext(nc) as tc:
        with (
            tc.tile_pool(name="sb", bufs=1) as sb,
            tc.tile_pool(name="ps", bufs=1, space="PSUM") as ps,
            tc.tile_pool(name="psm", bufs=2, space="PSUM") as psm,
            tc.tile_pool(name="dram", bufs=1, space="DRAM") as dram,
        )\
        :
            # ---- DMA in ----
            xw_t = [sb.tile([128, 768], F32R, name=f"xw{cb}", tag=f"xw{cb}") for cb in range(4)]
            bT_t = [sb.tile([128, H], F32, name=f"bT{gb}", tag=f"bT{gb}") for gb in range(2)]
            bkv_t = sb.tile([128, 256], F32, name="bkv", tag="bkv")
            qpb_t = sb.tile([128, 2], F32, name="qpb", tag="qpb")
            for cb in range(4):
                nc.sync.dma_start(xw_t[cb][:], xw[128 * cb:128 * (cb + 1), :])
            for gb in range(2):
                nc.sync.dma_start(bT_t[gb][:], biasT[128 * gb:128 * (gb + 1), :])
            nc.sync.dma_start(bkv_t[:], bkv[:, :])
            nc.sync.dma_start(qpb_t[:], qpb[:, :])

            # EBT = exp(biasT): [g, h] fp16
            ebt = [sb.tile([128, H], F16, name=f"ebt{gb}", tag=f"ebt{gb}") for gb in range(2)]
            for gb in range(2):
                nc.scalar.activation(
                    ebt[gb][:], bT_t[gb][:], mybir.ActivationFunctionType.Exp
                )

            # ---- QKV matmuls ----
            # kT/vT: out[g, (k|v)d] = sum_c x[c, g] * wkvT[c, :]
            kvt_ps = [ps.tile([128, 256], F32, name=f"kvt{gb}", tag=f"kvt{gb}") for gb in range(2)]
            for gb in range(2):
                for cb in range(4):
                    nc.tensor.matmul(
                        kvt_ps[gb][:],
                        xw_t[cb][:, 128 * gb:128 * (gb + 1)],
                        xw_t[cb][:, 384:640],
                        start=(cb == 0),
                        stop=(cb == 3),
                    )
            # q: out[d, h] = sum_c wqT[c, d] * x[c, h]
            q_ps = ps.tile([128, H], F32, name="q", tag="q")
            for cb in range(4):
                nc.tensor.matmul(
                    q_ps[:],
                    xw_t[cb][:, 256:384],
                    xw_t[cb][:, 0:256],
                    start=(cb == 0),
                    stop=(cb == 3),
                )

            # ---- bias add + cast ----
            # scaled k-bias: s * qkv_b[k-slice] replicated
            bks = sb.tile([128, 128], F32, name="bks", tag="bks")
            nc.scalar.activation(
                bks[:], bkv_t[:, 0:128],
                mybir.ActivationFunctionType.Copy, scale=SCALE,
            )
            # kh = s*k + s*bk ; vh = v + bv   (fp16, [g, d] layout)
            kh = [sb.tile([128, 128], F16, name=f"kh{gb}", tag=f"kh{gb}") for gb in range(2)]
            vh = [sb.tile([128, 128], F16, name=f"vh{gb}", tag=f"vh{gb}") for gb in range(2)]
            for gb in range(2):
                nc.vector.scalar_tensor_tensor(
                    kh[gb][:], kvt_ps[gb][:, 0:128], SCALE, bks[:],
                    op0=mybir.AluOpType.mult, op1=mybir.AluOpType.add,
                )
                nc.vector.tensor_tensor(
                    vh[gb][:], kvt_ps[gb][:, 128:256], bkv_t[:, 128:256],
                    op=mybir.AluOpType.add,
                )
            # qh = q + bq (per-partition bias) fp16 [d, h]
            qh = sb.tile([128, H], F16, name="qh", tag="qh")
            nc.scalar.activation(
                qh[:], q_ps[:], mybir.ActivationFunctionType.Identity,
                bias=qpb_t[:, 0:1],
            )

            # ---- power/column build (fp16, [g, d] tiles) ----
            # k powers: k2=kh^2 (ACT), k3=k2*kh, k4=k2^2 (ACT), k5=k3*k2
            # kv cols:  kv1=vh*kh, kv2=vh*k2, kv3=kv1*k2, kv4=kv2*k2, kv5=kv3*k2
            kpow = {}
            kvcol = {}
            ones_t = sb.tile([128, 128], F16, name="ones", tag="ones")
            nc.vector.memset(ones_t[:], 1.0)
            for gb in range(2):
                k2 = sb.tile([128, 128], F16, name=f"k2_{gb}", tag=f"k2_{gb}")
                k3 = sb.tile([128, 128], F16, name=f"k3_{gb}", tag=f"k3_{gb}")
                k4 = sb.tile([128, 128], F16, name=f"k4_{gb}", tag=f"k4_{gb}")
                k5 = sb.tile([128, 128], F16, name=f"k5_{gb}", tag=f"k5_{gb}")
                nc.scalar.activation(
                    k2[:], kh[gb][:], mybir.ActivationFunctionType.Square
                )
                nc.vector.tensor_tensor(
                    k3[:], k2[:], kh[gb][:], op=mybir.AluOpType.mult
                )
                nc.scalar.activation(
                    k4[:], k2[:], mybir.ActivationFunctionType.Square
                )
                nc.gpsimd.tensor_tensor(
                    k5[:], k3[:], k2[:], op=mybir.AluOpType.mult
                )
                kpow[gb] = [ones_t, kh[gb], k2, k3, k4, k5]

                kv1 = sb.tile([128, 128], F16, name=f"kv1_{gb}", tag=f"kv1_{gb}")
                kv2 = sb.tile([128, 128], F16, name=f"kv2_{gb}", tag=f"kv2_{gb}")
                kv3 = sb.tile([128, 128], F16, name=f"kv3_{gb}", tag=f"kv3_{gb}")
                kv4 = sb.tile([128, 128], F16, name=f"kv4_{gb}", tag=f"kv4_{gb}")
                kv5 = sb.tile([128, 128], F16, name=f"kv5_{gb}", tag=f"kv5_{gb}")
                nc.vector.tensor_tensor(
                    kv1[:], vh[gb][:], kh[gb][:], op=mybir.AluOpType.mult
                )
                nc.gpsimd.tensor_tensor(
                    kv2[:], vh[gb][:], k2[:], op=mybir.AluOpType.mult
                )
                nc.vector.tensor_tensor(
                    kv3[:], kv1[:], k2[:], op=mybir.AluOpType.mult
                )
                nc.gpsimd.tensor_tensor(
                    kv4[:], kv2[:], k2[:], op=mybir.AluOpType.mult
                )
                nc.vector.tensor_tensor(
                    kv5[:], kv3[:], k2[:], op=mybir.AluOpType.mult
                )
                kvcol[gb] = [vh[gb], kv1, kv2, kv3, kv4, kv5]

            # ---- EB matmuls + Horner (m = DEG .. 0) ----
            # Mv_m[d, h] = sum_g kvcol_m[g, d] * EBT[g, h]; Md_m likewise.
            # N-chain on DVE (reads Mv straight from PSUM);
            # D-chain on GPSIMD (needs SBUF, ACT evacuates Md).
            accN = sb.tile([128, H], F16, name="accN", tag="accN")
            accNf = sb.tile([128, H], F32, name="accNf", tag="accNf")
            accDf = sb.tile([128, H], F32, name="accDf", tag="accDf")
            tmpN = sb.tile([128, H], F16, name="tmpN", tag="tmpN")
            tmpD = sb.tile([128, H], F16, name="tmpD", tag="tmpD")
            accD_pp = [
                sb.tile([128, H], F16, name=f"accD{i}", tag=f"accD{i}")
                for i in range(2)
            ]

            accD = None
            for m in range(DEG, -1, -1):
                mv = psm.tile([128, H], F32, name="mv", tag="mv")
                md = psm.tile([128, H], F32, name="md", tag="md")
                for gb in range(2):
                    nc.tensor.matmul(
                        mv[:], kvcol[gb][m][:], ebt[gb][:],
                        start=(gb == 0), stop=(gb == 1),
                    )
                for gb in range(2):
                    nc.tensor.matmul(
                        md[:], kpow[gb][m][:], ebt[gb][:],
                        start=(gb == 0), stop=(gb == 1),
                    )
                # evacuate Md with the Chebyshev coefficient folded in
                mds = sb.tile([128, H], F16, name=f"mds{m % 3}", tag=f"mds{m % 3}")
                nc.scalar.activation(
                    mds[:], md[:], mybir.ActivationFunctionType.Copy,
                    scale=COEF[m],
                )
                if m == DEG:
                    nc.vector.tensor_scalar_mul(accN[:], mv[:], COEF[m])
                    accD = mds
                else:
                    outN = accNf if m == 0 else accN
                    outD = accDf if m == 0 else accD_pp[m % 2]
                    nc.vector.tensor_tensor(
                        tmpN[:], accN[:], qh[:], op=mybir.AluOpType.mult
                    )
                    nc.vector.scalar_tensor_tensor(
                        outN[:], mv[:], COEF[m], tmpN[:],
                        op0=mybir.AluOpType.mult, op1=mybir.AluOpType.add,
                    )
                    nc.gpsimd.tensor_tensor(
                        tmpD[:], accD[:], qh[:], op=mybir.AluOpType.mult
                    )
                    nc.gpsimd.tensor_tensor(
                        outD[:], tmpD[:], mds[:], op=mybir.AluOpType.add
                    )
                    accD = outD

            if stage == "horner":
                oh = sb.tile([128, H], F32, name="oh", tag="oh")
                nc.vector.tensor_copy(oh[:], accNf[:])
                nc.sync.dma_start(out[:, :], oh[:])
            if stage == "qkv":
                oq = sb.tile([128, H], F32, name="oq", tag="oq")
                nc.vector.tensor_copy(oq[:], qh[:])
                nc.sync.dma_start(out[:, :], oq[:])
            # ---- attnout = N / D ----
            recD = sb.tile([128, H], F32, name="recD", tag="recD")
            att = sb.tile([128, H], F32, name="att", tag="att")
            nc.vector.reciprocal_approx_fast(recD[:], accDf[:])
            nc.vector.tensor_tensor(
                att[:], accNf[:], recD[:], op=mybir.AluOpType.mult
            )

            if stage == "att":
                o16 = sb.tile([128, H], F32, name="o16", tag="o16")
                nc.vector.tensor_copy(o16[:], att[:])
                nc.sync.dma_start(out[:, :], o16[:])
            # ---- AllGather attnout within the 4-core batch group ----
            if stage == "full":
                _tail(nc, tc, sb, ps, dram, out, att, xw_t, qpb_t)
    nc.compile()
    return nc


def _tail(nc, tc, sb, ps, dram, out, att, xw_t, qpb_t):
    if True:
        if True:
            cc_in = dram.tile([DLOC, H], F32, name="cc_in")
            cc_out = dram.tile([C, H], F32, name="cc_out")
            nc.sync.dma_start(cc_in[:], att[:])
            nc.gpsimd.collective_compute(
                "AllGather",
                mybir.AluOpType.bypass,
                ins=[cc_in.opt()],
                outs=[cc_out.opt()],
                replica_groups=[[0, 1, 2, 3], [4, 5, 6, 7]],
            )

            # ---- proj: out[o, h] = sum_d pwT[d, o] * attnout[d, h] ----
            p_ps = ps.tile([128, H], F32, name="proj", tag="proj")
            afull = [sb.tile([128, H], F32R, name=f"af{db}", tag=f"af{db}") for db in range(4)]
            for db in range(4):
                nc.sync.dma_start(
                    afull[db][:], cc_out[128 * db:128 * (db + 1), :].bitcast(F32R)
                )
            for db in range(4):
                nc.tensor.matmul(
                    p_ps[:],
                    xw_t[db][:, 640:768],
                    afull[db][:],
                    start=(db == 0),
                    stop=(db == 3),
                )
            out_sb = sb.tile([128, H], F32, name="osb", tag="osb")
            nc.vector.tensor_scalar_add(out_sb[:], p_ps[:], qpb_t[:, 1:2])
            nc.sync.dma_start(out[:, :], out_sb[:])


_CACHED_NC = None


def _shard_inputs(x, qkv_w, qkv_b, proj_w, proj_b, rpb):
    x = np.ascontiguousarray(np.asarray(x, dtype=np.float32))
    qkv_w = np.asarray(qkv_w, dtype=np.float32)
    qkv_b = np.asarray(qkv_b, dtype=np.float32)
    proj_w = np.asarray(proj_w, dtype=np.float32)
    proj_b = np.asarray(proj_b, dtype=np.float32)
    rpb = np.asarray(rpb, dtype=np.float32)

    biasT = np.ascontiguousarray(
        rpb[RPI, 0].reshape(H, H).T.astype(np.float32)
    )
    in_maps = []
    for core in range(NCORES):
        b, j = divmod(core, GROUP)
        d0 = DLOC * j
        wq = qkv_w[d0:d0 + DLOC, :].T                      # [C, 128]
        wk = qkv_w[C + d0:C + d0 + DLOC, :].T              # [C, 128]
        wv = qkv_w[2 * C + d0:2 * C + d0 + DLOC, :].T      # [C, 128]
        pw = proj_w[d0:d0 + DLOC, :].T                     # [C, 128] rows o-slice
        xwm = np.ascontiguousarray(
            np.concatenate([x[b, :, :, 0], wq, wk, wv, pw], axis=1)  # [C, 768]
        )
        bkv = np.ascontiguousarray(
            np.broadcast_to(
                np.concatenate(
                    [qkv_b[C + d0:C + d0 + DLOC], qkv_b[2 * C + d0:2 * C + d0 + DLOC]]
                )[None, :],
                (128, 256),
            )
        ).astype(np.float32)
        qpb = np.ascontiguousarray(
            np.stack([qkv_b[d0:d0 + DLOC], proj_b[d0:d0 + DLOC]], axis=1)
        ).astype(np.float32)
        in_maps.append({
            "xw": xwm,
            "biasT": biasT,
            "bkv": bkv,
            "qpb": qpb,
        })
    return in_maps


def run(inputs, trace=False, **kwargs):
    global _CACHED_NC
    if _CACHED_NC is None:
        _CACHED_NC = build_nc()
    nc = _CACHED_NC
    in_maps = _shard_inputs(**inputs)
    res = run_bass_kernel_spmd(
        nc, in_maps, core_ids=list(range(NCORES)), trace=trace, **kwargs
    )
    out = np.empty((B, C, H, 1), dtype=np.float32)
    for core in range(NCORES):
        b, j = divmod(core, GROUP)
        out[b, DLOC * j:DLOC * (j + 1), :, 0] = res.results[core]["out"]
    return out, res


def kernel(**inputs):
    out, _ = run(inputs)
    return out


# revision 14
# speedup vs baseline: 1.1119x; 1.1119x over previous
"""Trainium2 Bass kernel for per-channel attention (nn_Attention_11690900979891).

Math (per batch b, channel d; H=256 positions, W=1):
    q,k,v = (qkv_w @ x_b + qkv_b) split              # each [512, 256]
    attn[h,g] = softmax_g(s*q[d,h]*k[d,g] + bias[h,g])
    attnout[d,h] = sum_g attn[h,g] * v[d,g]
    out_b = proj_w @ attnout + proj_b

Since |s*q*k| <= ~0.75, exp(s*q*k) is replaced by a degree-5 Chebyshev
polynomial; exp(z) ~= sum_m c_m z^m turns the softmax numerator/denominator
into dense GEMMs against EB = exp(bias):
    N[d,h] = sum_m c_m q[d,h]^m * (EB @ (v_d k_d^m))[h]
    D[d,h] = sum_m c_m q[d,h]^m * (EB @ (k_d^m))[h]
    attnout = N / D
so no transcendentals on the [256,256]-per-channel attention maps.

Sharding: core = (b, j); b = core//4, channels d in [128*j, 128*(j+1)).
Each core computes QKV + poly attention for its 128 channels, AllGathers
attnout within its 4-core batch group, then computes proj rows
[128*j : 128*(j+1)] of the output. Host only slices inputs / concatenates
outputs.
"""

import numpy as np

import concourse.bass as bass
import concourse.bacc as bacc
import concourse.mybir as mybir
from concourse import tile
from concourse.bass_utils import run_bass_kernel_spmd

F32 = mybir.dt.float32
F32R = mybir.dt.float32r
F16 = mybir.dt.float16

B, C, H = 2, 512, 256
NCORES = 8
GROUP = 4          # cores per batch
DLOC = C // GROUP  # 128 channels per core
SCALE = C ** -0.5
DEG = 4            # polynomial degree
POLY_A = 1.1       # fit domain [-A, A] for exp()

WS = 16
NTAB = (2 * WS - 1) ** 2


def _poly_coeffs():
    from numpy.polynomial import chebyshev as _ch
    c = _ch.Chebyshev.interpolate(np.exp, DEG, domain=[-POLY_A, POLY_A])
    return [float(v) for v in c.convert(kind=np.polynomial.Polynomial).coef]


COEF = _poly_coeffs()


def _rel_pos_index():
    coords = np.stack(
        np.meshgrid(np.arange(WS), np.arange(WS), indexing="ij"), 0
    ).reshape(2, -1)
    rel = coords[:, :, None] - coords[:, None, :]
    return np.mod(rel.transpose(1, 2, 0).sum(-1), NTAB).reshape(-1)


RPI = _rel_pos_index()


def build_nc(stage="full"):
    nc = bacc.Bacc(None, target_bir_lowering=False)

    xw = nc.declare_dram_parameter("xw", [C, 768], F32R, isOutput=False)
    biasT = nc.declare_dram_parameter("biasT", [H, H], F32, isOutput=False)
    bkv = nc.declare_dram_parameter("bkv", [128, 256], F32, isOutput=False)
    qpb = nc.declare_dram_parameter("qpb", [128, 2], F32, isOutput=False)
    out = nc.declare_dram_parameter("out", [DLOC, H], F32, isOutput=True)

    f32r = lambda ap: ap.bitcast(F32R)

    with tile.TileContext(nc) as tc:
        with (
            tc.tile_pool(name="sb", bufs=1) as sb,
            tc.tile_pool(name="ps", bufs=1, space="PSUM") as ps,
            tc.tile_pool(name="psm", bufs=2, space="PSUM") as psm,
            tc.tile_pool(name="dram", bufs=1, space="DRAM") as dram,
        )\
        :
            # ---- DMA in ----
            xw_t = [sb.tile([128, 768], F32R, name=f"xw{cb}", tag=f"xw{cb}") for cb in range(4)]
            bT_t = [sb.tile([128, H], F32, name=f"bT{gb}", tag=f"bT{gb}") for gb in range(2)]
            bkv_t = sb.tile([128, 256], F32, name="bkv", tag="bkv")
            qpb_t = sb.tile([128, 2], F32, name="qpb", tag="qpb")
            for cb in range(4):
                nc.sync.dma_start(xw_t[cb][:], xw[128 * cb:128 * (cb + 1), :])
            for gb in range(2):
                nc.sync.dma_start(bT_t[gb][:], biasT[128 * gb:128 * (gb + 1), :])
            nc.sync.dma_start(bkv_t[:], bkv[:, :])
            nc.sync.dma_start(qpb_t[:], qpb[:, :])

            # EBT = exp(biasT): [g, h] fp16
            ebt = [sb.tile([128, H], F16, name=f"ebt{gb}", tag=f"ebt{gb}") for gb in range(2)]
            for gb in range(2):
                nc.scalar.activation(
                    ebt[gb][:], bT_t[gb][:], mybir.ActivationFunctionType.Exp
                )

            # ---- QKV matmuls ----
            # kT/vT: out[g, (k|v)d] = sum_c x[c, g] * wkvT[c, :]
            kvt_ps = [ps.tile([128, 256], F32, name=f"kvt{gb}", tag=f"kvt{gb}") for gb in range(2)]
            for gb in range(2):
                for cb in range(4):
                    nc.tensor.matmul(
                        kvt_ps[gb][:],
                        xw_t[cb][:, 128 * gb:128 * (gb + 1)],
                        xw_t[cb][:, 384:640],
                        start=(cb == 0),
                        stop=(cb == 3),
                    )
            # q: out[d, h] = sum_c wqT[c, d] * x[c, h]
            q_ps = ps.tile([128, H], F32, name="q", tag="q")
            for cb in range(4):
                nc.tensor.matmul(
                    q_ps[:],
                    xw_t[cb][:, 256:384],
                    xw_t[cb][:, 0:256],
                    start=(cb == 0),
                    stop=(cb == 3),
                )

            # ---- bias add + cast ----
            # scaled k-bias: s * qkv_b[k-slice] replicated
            bks = sb.tile([128, 128], F32, name="bks", tag="bks")
            nc.scalar.activation(
                bks[:], bkv_t[:, 0:128],
                mybir.ActivationFunctionType.Copy, scale=SCALE,
            )
            # kh = s*k + s*bk ; vh = v + bv   (fp16, [g, d] layout)
            kh = [sb.tile([128, 128], F16, name=f"kh{gb}", tag=f"kh{gb}") for gb in range(2)]
            vh = [sb.tile([128, 128], F16, name=f"vh{gb}", tag=f"vh{gb}") for gb in range(2)]
            for gb in range(2):
                nc.vector.scalar_tensor_tensor(
                    kh[gb][:], kvt_ps[gb][:, 0:128], SCALE, bks[:],
                    op0=mybir.AluOpType.mult, op1=mybir.AluOpType.add,
                )
                nc.vector.tensor_tensor(
                    vh[gb][:], kvt_ps[gb][:, 128:256], bkv_t[:, 128:256],
                    op=mybir.AluOpType.add,
                )
            # qh = q + bq (per-partition bias) fp16 [d, h]
            qh = sb.tile([128, H], F16, name="qh", tag="qh")
            nc.scalar.activation(
                qh[:], q_ps[:], mybir.ActivationFunctionType.Identity,
                bias=qpb_t[:, 0:1],
            )

            # ---- power/column build (fp16, [g, d] tiles) ----
            # DVE + ACT only: concurrent GpSimd elementwise contends with DVE
            # on the shared SBUF port (exclusive lock), measured 2-3x slowdown.
            # k powers: k2=kh^2 (ACT), k3=k2*kh, k4=k2^2 (ACT)
            # kv cols:  kv1=vh*kh, kv2=vh*k2, kv3=kv1*k2, kv4=kv2*k2
            kpow = {}
            kvcol = {}
            ones_t = sb.tile([128, 128], F16, name="ones", tag="ones")
            nc.vector.memset(ones_t[:], 1.0)
            for gb in range(2):
                k2 = sb.tile([128, 128], F16, name=f"k2_{gb}", tag=f"k2_{gb}")
                k3 = sb.tile([128, 128], F16, name=f"k3_{gb}", tag=f"k3_{gb}")
                k4 = sb.tile([128, 128], F16, name=f"k4_{gb}", tag=f"k4_{gb}")
                nc.scalar.activation(
                    k2[:], kh[gb][:], mybir.ActivationFunctionType.Square
                )
                nc.vector.tensor_tensor(
                    k3[:], k2[:], kh[gb][:], op=mybir.AluOpType.mult
                )
                nc.scalar.activation(
                    k4[:], k2[:], mybir.ActivationFunctionType.Square
                )
                kpow[gb] = [ones_t, kh[gb], k2, k3, k4]

                kv1 = sb.tile([128, 128], F16, name=f"kv1_{gb}", tag=f"kv1_{gb}")
                kv2 = sb.tile([128, 128], F16, name=f"kv2_{gb}", tag=f"kv2_{gb}")
                kv3 = sb.tile([128, 128], F16, name=f"kv3_{gb}", tag=f"kv3_{gb}")
                kv4 = sb.tile([128, 128], F16, name=f"kv4_{gb}", tag=f"kv4_{gb}")
                nc.vector.tensor_tensor(
                    kv1[:], vh[gb][:], kh[gb][:], op=mybir.AluOpType.mult
                )
                nc.vector.tensor_tensor(
                    kv2[:], vh[gb][:], k2[:], op=mybir.AluOpType.mult
                )
                nc.vector.tensor_tensor(
                    kv3[:], kv1[:], k2[:], op=mybir.AluOpType.mult
                )
                nc.vector.tensor_tensor(
                    kv4[:], kv2[:], k2[:], op=mybir.AluOpType.mult
                )
                kvcol[gb] = [vh[gb], kv1, kv2, kv3, kv4]

            # ---- EB matmuls + Horner (m = DEG .. 0) ----
            # Mv_m[d, h] = sum_g kvcol_m[g, d] * EBT[g, h]; Md_m likewise.
            # ACT evacuates PSUM with the Chebyshev coefficient folded in;
            # both Horner chains run on DVE over fp16 SBUF tiles.
            accN = sb.tile([128, H], F16, name="accN", tag="accN")
            accNf = sb.tile([128, H], F32, name="accNf", tag="accNf")
            accDf = sb.tile([128, H], F32, name="accDf", tag="accDf")
            tmpN = sb.tile([128, H], F16, name="tmpN", tag="tmpN")
            tmpD = sb.tile([128, H], F16, name="tmpD", tag="tmpD")
            accD_pp = [
                sb.tile([128, H], F16, name=f"accD{i}", tag=f"accD{i}")
                for i in range(2)
            ]

            accD = None
            for m in range(DEG, -1, -1):
                mv = psm.tile([128, H], F32, name="mv", tag="mv")
                md = psm.tile([128, H], F32, name="md", tag="md")
                for gb in range(2):
                    nc.tensor.matmul(
                        mv[:], kvcol[gb][m][:], ebt[gb][:],
                        start=(gb == 0), stop=(gb == 1),
                    )
                for gb in range(2):
                    nc.tensor.matmul(
                        md[:], kpow[gb][m][:], ebt[gb][:],
                        start=(gb == 0), stop=(gb == 1),
                    )
                mds = sb.tile([128, H], F16, name=f"mds{m % 3}", tag=f"mds{m % 3}")
                nc.scalar.activation(
                    mds[:], md[:], mybir.ActivationFunctionType.Copy,
                    scale=COEF[m],
                )
                if m == DEG:
                    nc.vector.tensor_scalar_mul(accN[:], mv[:], COEF[m])
                    accD = mds
                else:
                    outN = accNf if m == 0 else accN
                    outD = accDf if m == 0 else accD_pp[m % 2]
                    nc.vector.tensor_tensor(
                        tmpN[:], accN[:], qh[:], op=mybir.AluOpType.mult
                    )
                    nc.vector.scalar_tensor_tensor(
                        outN[:], mv[:], COEF[m], tmpN[:],
                        op0=mybir.AluOpType.mult, op1=mybir.AluOpType.add,
                    )
                    nc.vector.tensor_tensor(
                        tmpD[:], accD[:], qh[:], op=mybir.AluOpType.mult
                    )
                    nc.vector.tensor_tensor(
                        outD[:], tmpD[:], mds[:], op=mybir.AluOpType.add
                    )
                    accD = outD

            if stage == "horner":
                oh = sb.tile([128, H], F32, name="oh", tag="oh")
                nc.vector.tensor_copy(oh[:], accNf[:])
                nc.sync.dma_start(out[:, :], oh[:])
            if stage == "qkv":
                oq = sb.tile([128, H], F32, name="oq", tag="oq")
                nc.vector.tensor_copy(oq[:], qh[:])
                nc.sync.dma_start(out[:, :], oq[:])
            # ---- attnout = N / D ----
            recD = sb.tile([128, H], F32, name="recD", tag="recD")
            att = sb.tile([128, H], F32, name="att", tag="att")
            nc.vector.reciprocal_approx_fast(recD[:], accDf[:])
            nc.vector.tensor_tensor(
                att[:], accNf[:], recD[:], op=mybir.AluOpType.mult
            )

            if stage == "att":
                o16 = sb.tile([128, H], F32, name="o16", tag="o16")
                nc.vector.tensor_copy(o16[:], att[:])
                nc.sync.dma_start(out[:, :], o16[:])
            # ---- AllGather attnout within the 4-core batch group ----
            if stage == "full":
                _tail(nc, tc, sb, ps, dram, out, att, xw_t, qpb_t)
    nc.compile()
    return nc


def _tail(nc, tc, sb, ps, dram, out, att, xw_t, qpb_t):
    if True:
        if True:
            cc_in = dram.tile([DLOC, H], F32, name="cc_in")
            cc_out = dram.tile([C, H], F32, name="cc_out")
            nc.sync.dma_start(cc_in[:], att[:])
            nc.gpsimd.collective_compute(
                "AllGather",
                mybir.AluOpType.bypass,
                ins=[cc_in.opt()],
                outs=[cc_out.opt()],
                replica_groups=[[0, 1, 2, 3], [4, 5, 6, 7]],
            )

            # ---- proj: out[o, h] = sum_d pwT[d, o] * attnout[d, h] ----
            p_ps = ps.tile([128, H], F32, name="proj", tag="proj")
            afull = [sb.tile([128, H], F32R, name=f"af{db}", tag=f"af{db}") for db in range(4)]
            for db in range(4):
                nc.sync.dma_start(
                    afull[db][:], cc_out[128 * db:128 * (db + 1), :].bitcast(F32R)
                )
            for db in range(4):
                nc.tensor.matmul(
                    p_ps[:],
                    xw_t[db][:, 640:768],
                    afull[db][:],
                    start=(db == 0),
                    stop=(db == 3),
                )
            out_sb = sb.tile([128, H], F32, name="osb", tag="osb")
            nc.vector.tensor_scalar_add(out_sb[:], p_ps[:], qpb_t[:, 1:2])
            nc.sync.dma_start(out[:, :], out_sb[:])


_CACHED_NC = None


def _shard_inputs(x, qkv_w, qkv_b, proj_w, proj_b, rpb):
    x = np.ascontiguousarray(np.asarray(x, dtype=np.float32))
    qkv_w = np.asarray(qkv_w, dtype=np.float32)
    qkv_b = np.asarray(qkv_b, dtype=np.float32)
    proj_w = np.asarray(proj_w, dtype=np.float32)
    proj_b = np.asarray(proj_b, dtype=np.float32)
    rpb = np.asarray(rpb, dtype=np.float32)

    biasT = np.ascontiguousarray(
        rpb[RPI, 0].reshape(H, H).T.astype(np.float32)
    )
    in_maps = []
    for core in range(NCORES):
        b, j = divmod(core, GROUP)
        d0 = DLOC * j
        wq = qkv_w[d0:d0 + DLOC, :].T                      # [C, 128]
        wk = qkv_w[C + d0:C + d0 + DLOC, :].T              # [C, 128]
        wv = qkv_w[2 * C + d0:2 * C + d0 + DLOC, :].T      # [C, 128]
        pw = proj_w[d0:d0 + DLOC, :].T                     # [C, 128] rows o-slice
        xwm = np.ascontiguousarray(
            np.concatenate([x[b, :, :, 0], wq, wk, wv, pw], axis=1)  # [C, 768]
        )
        bkv = np.ascontiguousarray(
            np.broadcast_to(
                np.concatenate(
                    [qkv_b[C + d0:C + d0 + DLOC], qkv_b[2 * C + d0:2 * C + d0 + DLOC]]
                )[None, :],
                (128, 256),
            )
        ).astype(np.float32)
        qpb = np.ascontiguousarray(
            np.stack([qkv_b[d0:d0 + DLOC], proj_b[d0:d0 + DLOC]], axis=1)
        ).astype(np.float32)
        in_maps.append({
            "xw": xwm,
            "biasT": biasT,
            "bkv": bkv,
            "qpb": qpb,
        })
    return in_maps


def run(inputs, trace=False, **kwargs):
    global _CACHED_NC
    if _CACHED_NC is None:
        _CACHED_NC = build_nc()
    nc = _CACHED_NC
    in_maps = _shard_inputs(**inputs)
    res = run_bass_kernel_spmd(
        nc, in_maps, core_ids=list(range(NCORES)), trace=trace, **kwargs
    )
    out = np.empty((B, C, H, 1), dtype=np.float32)
    for core in range(NCORES):
        b, j = divmod(core, GROUP)
        out[b, DLOC * j:DLOC * (j + 1), :, 0] = res.results[core]["out"]
    return out, res


def kernel(**inputs):
    out, _ = run(inputs)
    return out
